# revision 6
# baseline (speedup 1.0000x reference)
"""GroupEmbedding Trainium2 kernel v4 (8 NeuronCores, data-parallel groups).

Key mechanism: the batched-gather ucode instruction (InstDMAGatherAnt via
nc.gpsimd.dma_gather, ~0.34ns/descriptor desc-gen) instead of per-row
indirect DMAs (~1us fixed cost each -- the 6.5ms baseline's bottleneck).
dma_gather's int16 index limit (32768 rows) cannot address the 100k-row
tables directly, so we gather QUADS: the table is viewed as [V/4, 4*D];
idx = row//4 <= 24999 fits int16 and each 512B descriptor fetches 4
consecutive rows. The needed row (row%4, host-known) is selected on-chip by
folding a one-hot mask into the count multiply; the 4x extra elements are
summed away by the same contiguous tree-reduction that does the M-sum.
Tables are bf16 (rel-err ~6e-3, inside the 2e-2 gate).
"""
from contextlib import ExitStack

import numpy as np

import concourse.bass as bass
import concourse.bacc as bacc
import concourse.mybir as mybir
import concourse.tile as tile
from concourse.bass import IndirectOffsetOnAxis
from concourse.bass_utils import run_bass_kernel_spmd

G, U, M = 4096, 50, 20
D = 64
V = 100000
Q = V // 4                   # 25000 quads (int16-addressable)
FACTOR = 0.5
NCORES = 8
GPC = G // NCORES            # 512 groups per core
NWAVE = GPC // 128           # 4 waves of 128 groups
UC = 2                       # users per item chunk
CI = UC * M                  # 40 tokens per chunk (m-major: j = m*UC + u)
NCHUNK = U // UC             # 20 chunks per wave
NIDX = 128 * CI              # gather indices per chunk
NI16 = NIDX // 16            # idx columns in the 16-partition wrap
UH = U // 2                  # 25 users per u-path half
NIDXU = 128 * UH
NI16U = NIDXU // 16

f32 = mybir.dt.float32
bf16 = mybir.dt.bfloat16
i32 = mybir.dt.int32
i16 = mybir.dt.int16

_CACHE = {}


def _ensure_ntff_hook():
    try:
        import antenv.axon_hooks  # noqa: F401
        return
    except ImportError:
        pass
    import contextlib
    import ctypes
    import sys
    import types

    mod = types.ModuleType("antenv.axon_hooks")
    holder = {}
    mod.set_axon_ntff_profile_hook = lambda h: holder.__setitem__("h", h)
    mod.get_axon_ntff_profile_hook = lambda: holder.get("h")
    try:
        lib = ctypes.CDLL("/opt/axon/libaxon_pjrt.so")
        if hasattr(lib, "axon_start_nrt_profile"):
            lib.axon_start_nrt_profile.argtypes = [
                ctypes.POINTER(ctypes.c_int64), ctypes.c_size_t]
            lib.axon_start_nrt_profile.restype = ctypes.c_int64
            lib.axon_stop_nrt_profile.argtypes = [ctypes.c_char_p]
            lib.axon_stop_nrt_profile.restype = ctypes.c_int64

            @contextlib.contextmanager
            def _hook(output_dir, device_ids):
                import jax
                jax.devices()
                if device_ids:
                    ids = (ctypes.c_int64 * len(device_ids))(*device_ids)
                    rc = lib.axon_start_nrt_profile(ids, len(device_ids))
                else:
                    rc = lib.axon_start_nrt_profile(None, 0)
                if rc != 0:
                    raise RuntimeError(f"axon_start_nrt_profile rc={rc}")
                try:
                    yield
                finally:
                    n = lib.axon_stop_nrt_profile(str(output_dir).encode())
                    print(f"ntff profile: {n} file(s) -> {output_dir}",
                          file=sys.stderr)

            holder["h"] = _hook
    except OSError:
        pass
    import antenv
    sys.modules["antenv.axon_hooks"] = mod
    antenv.axon_hooks = mod


def _build_program():
    nc = bacc.Bacc("TRN2", target_bir_lowering=False, debug=False,
                   num_devices=NCORES, dynamic_dma_scratch_size=1 << 15,
                   num_swdge_queues=4)
    itemq = nc.dram_tensor("itemq", [Q, 4 * D], bf16,
                           kind="ExternalInput").ap()
    usimq = nc.dram_tensor("usimq", [Q, 8 * D], bf16,
                           kind="ExternalInput").ap()
    item_i16 = nc.dram_tensor("item_i16", [NWAVE, NCHUNK, 128, NI16], i16,
                              kind="ExternalInput").ap()
    cm4 = nc.dram_tensor("cm4", [NWAVE, NCHUNK, 128, CI, 8], bf16,
                         kind="ExternalInput").ap()
    u_i16 = nc.dram_tensor("u_i16", [NWAVE, 2, 128, NI16U], i16,
                           kind="ExternalInput").ap()
    um4 = nc.dram_tensor("um4", [NWAVE, 2, 128, UH, 8], bf16,
                         kind="ExternalInput").ap()
    tq_idx = nc.dram_tensor("tq_idx", [NWAVE, 128, 8], i16,
                            kind="ExternalInput").ap()
    tm4 = nc.dram_tensor("tm4", [NWAVE, 128, 8], bf16,
                         kind="ExternalInput").ap()
    out = nc.dram_tensor("out", [GPC, D], f32, kind="ExternalOutput").ap()

    mult = mybir.AluOpType.mult
    add = mybir.AluOpType.add

    def tt(eng, o, a, b, op):
        eng.tensor_tensor(out=o, in0=a, in1=b, op=op)

    with tile.TileContext(nc) as tc:
        with ExitStack() as ctx:
            p_gq = ctx.enter_context(tc.tile_pool(name="gq", bufs=3))
            p_ix = ctx.enter_context(tc.tile_pool(name="ix", bufs=3))
            p_uq = ctx.enter_context(tc.tile_pool(name="uq", bufs=1))
            p_ub = ctx.enter_context(tc.tile_pool(name="ub", bufs=2))
            p_sm = ctx.enter_context(tc.tile_pool(name="sm", bufs=2))

            for w in range(NWAVE):
                ub = p_ub.tile([128, U, D], bf16)
                for c in range(NCHUNK):
                    ixt = p_ix.tile([128, NI16], i16, tag="ix")
                    nc.sync.dma_start(ixt[:], item_i16[w, c])
                    cmt = p_ix.tile([128, CI, 8], bf16, tag="cm")
                    nc.sync.dma_start(cmt[:], cm4[w, c])
                    gq = p_gq.tile([128, CI, 4 * D], bf16)
                    for s in range(CI // 8):
                        nc.gpsimd.dma_gather(
                            out_ap=gq[:, 8 * s:8 * (s + 1), :],
                            in_ap=itemq[:],
                            idxs_ap=ixt[:, 64 * s:64 * (s + 1)],
                            num_idxs=1024, num_idxs_reg=1024,
                            elem_size=4 * D, queue_num=s % 4)
                    # fused row-select + count multiply (one-hot count mask)
                    gv = gq[:].rearrange("p j (h s t) -> p (j h) s t",
                                         h=4, t=2)
                    cv = cmt[:].rearrange("p j (h t) -> p (j h) t",
                                          h=4).unsqueeze(2).to_broadcast(
                        [128, CI * 4, D // 2, 2])
                    tt(nc.vector, gv, gv, cv, mult)
                    # tree-reduce over m: 20 -> 10 -> 5 -> 4 -> 2 -> 1
                    gm = gq[:].rearrange("p (m u) e -> p m (u e)", m=M)
                    tt(nc.vector, gm[:, 0:10, :], gm[:, 0:10, :],
                       gm[:, 10:20, :], add)
                    tt(nc.vector, gm[:, 0:5, :], gm[:, 0:5, :],
                       gm[:, 5:10, :], add)
                    tt(nc.vector, gm[:, 0:1, :], gm[:, 0:1, :],
                       gm[:, 4:5, :], add)
                    tt(nc.vector, gm[:, 0:2, :], gm[:, 0:2, :],
                       gm[:, 2:4, :], add)
                    tt(nc.vector, gm[:, 0:1, :], gm[:, 0:1, :],
                       gm[:, 1:2, :], add)
                    # collapse the 4 quad-halves; last add lands in ub
                    hq = gq[:, 0:UC, :].rearrange("p j (h d) -> p j h d", h=4)
                    tt(nc.vector, hq[:, :, 0:2, :], hq[:, :, 0:2, :],
                       hq[:, :, 2:4, :], add)
                    tt(nc.vector, ub[:, c * UC:(c + 1) * UC, :],
                       hq[:, :, 0, :], hq[:, :, 1, :], add)
                # user/similarity path: two halves of 25 users
                uhsel = []
                for hf in range(2):
                    uxt = p_ix.tile([128, NI16U], i16, tag=f"ux{hf}")
                    nc.sync.dma_start(uxt[:], u_i16[w, hf])
                    umt = p_ix.tile([128, UH, 8], bf16, tag=f"um{hf}")
                    nc.sync.dma_start(umt[:], um4[w, hf])
                    uq = p_uq.tile([128, UH, 8 * D], bf16, tag=f"uq{hf}")
                    for s in range(UH // 5):
                        nc.gpsimd.dma_gather(
                            out_ap=uq[:, 5 * s:5 * (s + 1), :],
                            in_ap=usimq[:],
                            idxs_ap=uxt[:, 40 * s:40 * (s + 1)],
                            num_idxs=640, num_idxs_reg=640,
                            elem_size=8 * D, queue_num=s % 4)
                    uv = uq[:].rearrange("p u (h s t) -> p (u h) s t",
                                         h=4, t=2)
                    umv = umt[:].rearrange("p u (h t) -> p (u h) t",
                                           h=4).unsqueeze(2).to_broadcast(
                        [128, UH * 4, D, 2])
                    tt(nc.vector, uv, uv, umv, mult)
                    hu = uq[:].rearrange("p u (h e) -> p u h e", h=4)
                    tt(nc.vector, hu[:, :, 0:2, :], hu[:, :, 0:2, :],
                       hu[:, :, 2:4, :], add)
                    tt(nc.vector, hu[:, :, 0:1, :], hu[:, :, 0:1, :],
                       hu[:, :, 1:2, :], add)
                    uhsel.append(hu[:, :, 0, :])  # [128, UH, 2D] strided
                # target row: quad-gather one offset/partition + select
                tqt = p_ix.tile([128, 8], i16, tag="tq")
                nc.sync.dma_start(tqt[:], tq_idx[w])
                tmt = p_ix.tile([128, 8], bf16, tag="tm")
                nc.sync.dma_start(tmt[:], tm4[w])
                tgq = p_sm.tile([128, 1, 8 * D], bf16, tag="tgq")
                nc.gpsimd.dma_gather(
                    out_ap=tgq[:], in_ap=usimq[:], idxs_ap=tqt[:],
                    num_idxs=128, num_idxs_reg=128, elem_size=8 * D)
                tv = tgq[:].rearrange("p o (h s t) -> p (o h) s t", h=4, t=2)
                tmv = tmt[:].rearrange("p (h t) -> p h t", h=4).unsqueeze(
                    2).to_broadcast([128, 4, D, 2])
                tt(nc.vector, tv, tv, tmv, mult)
                th = tgq[:].rearrange("p o (h e) -> p (o h) e", h=4)
                tt(nc.vector, th[:, 0:2, :], th[:, 0:2, :], th[:, 2:4, :],
                   add)
                tt(nc.vector, th[:, 0:1, :], th[:, 0:1, :], th[:, 1:2, :],
                   add)
                # similarity weights + personalization + group reduce
                res = p_sm.tile([128, D], f32, tag="res")
                for hf in range(2):
                    us = uhsel[hf]
                    sg = us[:, :, D:2 * D]
                    tt(nc.vector, sg, sg,
                       tgq[:, 0:1, D:2 * D].to_broadcast([128, UH, D]), mult)
                    simw = p_sm.tile([128, UH], f32, tag=f"sw{hf}")
                    nc.vector.reduce_sum(out=simw[:], in_=sg,
                                         axis=mybir.AxisListType.X)
                    nc.vector.tensor_scalar_mul(out=simw[:], in0=simw[:],
                                                scalar1=FACTOR)
                    ubh = ub[:, hf * UH:(hf + 1) * UH, :]
                    tt(nc.vector, ubh, ubh, us[:, :, 0:D], mult)
                    tt(nc.vector, ubh, ubh,
                       simw[:].unsqueeze(2).to_broadcast([128, UH, D]), mult)
                nc.vector.reduce_sum(out=res[:], in_=ub[:].transpose([0, 2, 1]),
                                     axis=mybir.AxisListType.X)
                nc.sync.dma_start(out[w * 128:(w + 1) * 128, :], res[:])
    nc.finalize()
    return nc


def _wrap16(q):
    """[..., 128, N] int16 slot-ordered values -> [..., 128, N*128//16]
    16-partition-wrapped index tiles (replicated to all 8 core groups)."""
    lead = q.shape[:-2]
    n = q.shape[-1]
    ni16 = 128 * n // 16
    # flat slot i = j*128 + p  ->  F[..., i]
    f = np.swapaxes(q, -1, -2).reshape(lead + (128 * n,))
    w = np.swapaxes(f.reshape(lead + (ni16, 16)), -1, -2)  # [..., 16, ni16]
    w = np.broadcast_to(w[..., None, :, :], lead + (8, 16, ni16))
    return np.ascontiguousarray(w.reshape(lead + (128, ni16)))


def _onehot8(h, val):
    """one-hot over 4 with value `val`, duplicated pairs -> [..., 8] bf16."""
    import ml_dtypes
    oh = np.zeros(h.shape + (4,), dtype=np.float32)
    np.put_along_axis(oh, h[..., None].astype(np.int64), val[..., None], -1)
    oh = oh.astype(ml_dtypes.bfloat16)
    return np.ascontiguousarray(
        np.stack([oh, oh], axis=-1).reshape(h.shape + (8,)))


def _prep_in_maps(group_user, behavior_ids, behavior_counts, target_user,
                  similarity_vec, user_emb_w, item_emb_w):
    import ml_dtypes
    bf = ml_dtypes.bfloat16

    itemq = np.ascontiguousarray(item_emb_w, dtype=np.float32).astype(
        bf).reshape(Q, 4 * D)
    usimq = np.concatenate(
        [np.asarray(user_emb_w, np.float32),
         np.asarray(similarity_vec, np.float32)], axis=1).astype(bf).reshape(
        Q, 8 * D)

    # item tokens, m-major within chunk: j = m*UC + u_local
    r = np.asarray(behavior_ids, np.int64).reshape(
        NCORES, NWAVE, 128, NCHUNK, UC, M)
    r = np.ascontiguousarray(r.transpose(0, 1, 3, 2, 5, 4)).reshape(
        NCORES, NWAVE, NCHUNK, 128, CI)
    cc = np.asarray(behavior_counts, np.float32).reshape(
        NCORES, NWAVE, 128, NCHUNK, UC, M)
    cc = np.ascontiguousarray(cc.transpose(0, 1, 3, 2, 5, 4)).reshape(
        NCORES, NWAVE, NCHUNK, 128, CI)
    item_i16 = _wrap16((r // 4).astype(np.int16))
    cm4 = _onehot8(r % 4, cc)

    gu = np.asarray(group_user, np.int64).reshape(
        NCORES, NWAVE, 128, 2, UH).transpose(0, 1, 3, 2, 4)
    gu = np.ascontiguousarray(gu)  # [K, W, 2, 128, UH]
    u_i16 = _wrap16((gu // 4).astype(np.int16))
    um4 = _onehot8(gu % 4, np.ones_like(gu, dtype=np.float32))

    t = np.asarray(target_user, np.int64).reshape(NCORES, NWAVE, 128, 1)
    tq_idx = _wrap16((t // 4).astype(np.int16))
    tm4 = _onehot8((t % 4)[..., 0], np.ones(t.shape[:-1], np.float32))

    in_maps = []
    for k in range(NCORES):
        in_maps.append({
            "itemq": itemq,
            "usimq": usimq,
            "item_i16": np.ascontiguousarray(item_i16[k]),
            "cm4": np.ascontiguousarray(cm4[k]),
            "u_i16": np.ascontiguousarray(u_i16[k]),
            "um4": np.ascontiguousarray(um4[k]),
            "tq_idx": np.ascontiguousarray(tq_idx[k]),
            "tm4": np.ascontiguousarray(tm4[k]),
        })
    return in_maps


def kernel(group_user, behavior_ids, behavior_counts, target_user,
           similarity_vec, user_emb_w, item_emb_w, _trace=False):
    _ensure_ntff_hook()
    if "nc" not in _CACHE:
        _CACHE["nc"] = _build_program()
    nc = _CACHE["nc"]
    in_maps = _prep_in_maps(group_user, behavior_ids, behavior_counts,
                            target_user, similarity_vec, user_emb_w, item_emb_w)
    r = run_bass_kernel_spmd(nc, in_maps, core_ids=list(range(NCORES)),
                             trace=_trace)
    out = np.concatenate([r.results[k]["out"] for k in range(NCORES)], axis=0)
    _CACHE["last_result"] = r
    return out
